# revision 2
# baseline (speedup 1.0000x reference)
"""Trainium2 kernel for nn_COSSIMMLP (gnn_message_passing).

reference semantics:
    src = prop_state[b, mask[...,0]]; dst = prop_state[b, mask[...,1]]
    vals = sigmoid(cossim(src, dst))          # [B, E]
    adj[b, i, j] = vals; adj[b, j, i] = vals  # dense [B, N, N]

Every scatter write at position (r, c) carries the identical value
sigmoid(cos(s_r, s_c)) (reversed edges / duplicate edges give bit-identical
f32 values in the reference), so the output is exactly

    adj = edge_mask * sigmoid(S_hat @ S_hat.T)

with S_hat the eps-clamp-normalized rows and edge_mask a 0/1 indicator of
(i,j)|(j,i) edge positions.  The mask depends only on the integer index
tensor, so the host precomputes it as a bf16 0/1 matrix; all float math
(normalization, gram matmul, sigmoid, masking) runs on device.

Sharding: 8 cores = 4 batches x 2 row-halves.  Each core computes a
[2048, 4096] slab of one batch's adjacency.  Per-core node order is rolled
by the row offset so that a single SPMD program (rows = local nodes 0..2047)
serves all cores; the host un-rolls output columns.
"""

import numpy as np
import ml_dtypes

B, N, D, E = 4, 4096, 256, 131072
NH = N // 2          # rows per core
P = 128              # partitions
NT = N // P          # 32 node tiles
MT = NH // P         # 16 row tiles per core
EPS = 1e-8

_prog = None


def _build_program():
    import concourse.tile as tile
    from concourse import bacc, mybir
    from concourse.masks import make_identity

    f32 = mybir.dt.float32
    f16 = mybir.dt.float16
    bf16 = mybir.dt.bfloat16
    ACT = mybir.ActivationFunctionType

    nc = bacc.Bacc("TRN2", target_bir_lowering=False, debug=False)
    s_in = nc.dram_tensor("s", [N, D], f32, kind="ExternalInput")
    m_in = nc.dram_tensor("m", [NH, N], bf16, kind="ExternalInput")
    out = nc.dram_tensor("out", [NH, N], f32, kind="ExternalOutput")

    with tile.TileContext(nc) as tc:
        with tc.tile_pool(name="const", bufs=1) as cpool:
            ident = cpool.tile([P, P], f16)
            make_identity(nc, ident[:])
            # S_hat.T, split into the two 128-dim chunks of D=256
            st0 = cpool.tile([P, N], f16)
            st1 = cpool.tile([P, N], f16)

            # ---- phase A: load, normalize, transpose ----
            with (
                tc.tile_pool(name="prep", bufs=1) as prep,
                tc.tile_pool(name="prep_sc", bufs=2) as prep_sc,
                tc.tile_pool(name="prep_ps", bufs=2, space="PSUM") as prep_ps,
            ):
                s_sb = prep.tile([P, NT, D], f32)
                nc.sync.dma_start(
                    out=s_sb[:, :, :],
                    in_=s_in.rearrange("(t p) d -> p t d", p=P),
                )
                nsq = prep.tile([P, NT], f32)
                for t in range(NT):
                    sq = prep_sc.tile([P, D], f32, tag="sq")
                    nc.scalar.activation(
                        out=sq[:],
                        in_=s_sb[:, t, :],
                        func=ACT.Square,
                        accum_out=nsq[:, t : t + 1],
                    )
                nrm = prep.tile([P, NT], f32)
                nc.scalar.activation(out=nrm[:], in_=nsq[:], func=ACT.Sqrt)
                nc.vector.tensor_scalar_max(out=nrm[:], in0=nrm[:], scalar1=EPS)
                inv = prep.tile([P, NT], f32)
                nc.vector.reciprocal(out=inv[:], in_=nrm[:])
                shat = prep.tile([P, NT, D], f16)
                for t in range(NT):
                    nc.scalar.activation(
                        out=shat[:, t, :],
                        in_=s_sb[:, t, :],
                        func=ACT.Copy,
                        scale=inv[:, t : t + 1],
                    )
                for t in range(NT):
                    for dch, std in ((0, st0), (1, st1)):
                        pt = prep_ps.tile([P, P], f16, tag="tp")
                        nc.tensor.transpose(
                            pt[:], shat[:, t, dch * P : (dch + 1) * P], ident[:]
                        )
                        nc.vector.tensor_copy(
                            out=std[:, t * P : (t + 1) * P], in_=pt[:]
                        )

            # ---- phase B: gram matmul -> sigmoid -> mask -> store ----
            with (
                tc.tile_pool(name="mrow", bufs=3) as mrow,
                tc.tile_pool(name="sigp", bufs=3) as sigp,
                tc.tile_pool(name="outp", bufs=3) as outp,
                tc.tile_pool(name="mmps", bufs=2, space="PSUM") as mmps,
            ):
                for m in range(MT):
                    msk = mrow.tile([P, N], bf16, tag="msk")
                    nc.scalar.dma_start(out=msk[:], in_=m_in[m * P : (m + 1) * P, :])
                    ot = outp.tile([P, N], f32, tag="ot")
                    for g in range(2):
                        ps = mmps.tile([P, 2048], f32, tag="ps")
                        for k, stk in ((0, st0), (1, st1)):
                            lhsT = stk[:, m * P : (m + 1) * P]
                            for q in range(4):
                                nc.tensor.matmul(
                                    ps[:, q * 512 : (q + 1) * 512],
                                    lhsT=lhsT,
                                    rhs=stk[:, g * 2048 + q * 512 : g * 2048 + (q + 1) * 512],
                                    start=(k == 0),
                                    stop=(k == 1),
                                )
                        sg = sigp.tile([P, 2048], f32, tag="sg")
                        nc.scalar.activation(out=sg[:], in_=ps[:], func=ACT.Sigmoid)
                        nc.vector.tensor_tensor(
                            out=ot[:, g * 2048 : (g + 1) * 2048],
                            in0=sg[:],
                            in1=msk[:, g * 2048 : (g + 1) * 2048],
                            op=mybir.AluOpType.mult,
                        )
                    nc.sync.dma_start(out=out[m * P : (m + 1) * P, :], in_=ot[:])

    nc.compile()
    return nc


def _host_prep(prop_state, mask):
    prop = np.ascontiguousarray(np.asarray(prop_state), dtype=np.float32)
    mk = np.asarray(mask)
    i = mk[..., 0].astype(np.int64)
    j = mk[..., 1].astype(np.int64)
    one_bf16 = np.uint16(0x3F80)
    adjmask = np.zeros((B, N * N), dtype=np.uint16)
    for b in range(B):
        flat = adjmask[b]
        flat[i[b] * N + j[b]] = one_bf16
        flat[j[b] * N + i[b]] = one_bf16
    adjmask = adjmask.reshape(B, N, N)

    in_maps = []
    for c in range(8):
        b, h = divmod(c, 2)
        r = h * NH
        s_roll = prop[b] if r == 0 else np.roll(prop[b], -r, axis=0)
        msh = adjmask[b][r : r + NH]
        if r:
            msh = np.roll(msh, -r, axis=1)
        in_maps.append(
            {
                "s": np.ascontiguousarray(s_roll),
                "m": np.ascontiguousarray(msh).view(ml_dtypes.bfloat16),
            }
        )
    return in_maps


def _assemble(results):
    outf = np.empty((B, N, N), dtype=np.float32)
    for c in range(8):
        b, h = divmod(c, 2)
        r = h * NH
        o = results[c]["out"]
        outf[b, r : r + NH, :] = o if r == 0 else np.roll(o, r, axis=1)
    return outf


def kernel(prop_state, mask):
    from concourse.bass_utils import run_bass_kernel_spmd

    global _prog
    if _prog is None:
        _prog = _build_program()
    in_maps = _host_prep(prop_state, mask)
    res = run_bass_kernel_spmd(_prog, in_maps, core_ids=list(range(8)))
    return _assemble(res.results)
